# revision 1
# baseline (speedup 1.0000x reference)
"""Trainium2 Bass kernel for MultiHeadAttention with RoPE.

Problem: B=2, L=2048, d_model=1024, 16 heads, d_k=64, fp32 in/out.

Sharding (8 cores): tensor-parallel over heads — core c owns heads
{2c, 2c+1}, i.e. a 128-wide slice of the projection output dims.  Every
core reads the full q/k/v activations (transposed + bf16 on host), its
own 128-row slice of Wq/Wk/Wv (pre-transposed) and the matching 128
columns of Wo.  Each core computes its heads' attention output and the
partial d_model-sized output projection; the host sums the 8 partials
and adds bo.

Per-core pipeline (all matmuls bf16, fp32 PSUM accumulation):
  1. QKV projections:  qh.T = WqT.T @ q.T  laid out [128 head-dims, 4096 tok]
  2. RoPE on q,k via partition-shifted DMA copy + 3 DVE ops; the 1/sqrt(dk)
     scale and the rotate-half sign are folded into host-built cos/sin tables
  3. scores.T tiles [kt 128, qt 512] = kh'' (stationary, K=64) @ qh''
  4. exp on ScalarE (no max-subtract: scores ~ N(0,1), fp32 exp is safe),
     output bf16 -> SBUF
  5. ctx accumulation [65, qt]: stationary vh_aug [kt, 64 dims + ones col]
     -> row 64 accumulates the softmax denominator for free
  6. normalize via reciprocal + PE broadcast + DVE multiply (flash-style
     deferred normalization: applied to ctx, not to the 16.8M scores)
  7. out_partial[tok, 1024] = ctx (stationary) @ WoT slice
"""

import os
import numpy as np
import ml_dtypes

import concourse.bass as bass
import concourse.mybir as mybir
import concourse.tile as tile
from concourse import bacc
from concourse.bass_utils import run_bass_kernel_spmd

BF = mybir.dt.bfloat16
F32 = mybir.dt.float32
AF = mybir.ActivationFunctionType

NCORES = 8
B = 2
L = 2048
D = 1024          # d_model
H = 16            # heads
DK = 64           # head dim
HPC = H // NCORES  # heads per core = 2
PD = HPC * DK      # projection dims per core = 128
TOK = B * L        # 4096 tokens
P = 128

ROPE_BASE = 10000.0


def build_nc(debug_dumps=False):
    """Build the single-core Bass program (SPMD: same program, per-core data)."""
    from contextlib import ExitStack

    nc = bacc.Bacc("TRN2", target_bir_lowering=False, debug=False)
    dbg = {}
    if debug_dumps:
        for nm, shp, dt in [
            ("dbg_qq", [P, TOK], BF), ("dbg_kk", [P, TOK], BF),
            ("dbg_vh", [P, TOK], BF), ("dbg_vaug", [P, 64 * P], BF),
            ("dbg_exp", [P, 1024], BF), ("dbg_cp", [P, 1024], F32),
            ("dbg_rec", [2, 1024], F32), ("dbg_bcs", [P, 1024], F32),
            ("dbg_rsum", [2, 1024], F32),
            ("dbg_ctx", [P, TOK], BF),
        ]:
            dbg[nm] = nc.dram_tensor(nm, shp, dt, kind="ExternalOutput").ap()

    # ---- DRAM I/O ----
    qT = nc.dram_tensor("qT", [D, TOK], BF, kind="ExternalInput").ap()
    kT = nc.dram_tensor("kT", [D, TOK], BF, kind="ExternalInput").ap()
    vT = nc.dram_tensor("vT", [D, TOK], BF, kind="ExternalInput").ap()
    wqT = nc.dram_tensor("wqT", [D, PD], BF, kind="ExternalInput").ap()
    wkT = nc.dram_tensor("wkT", [D, PD], BF, kind="ExternalInput").ap()
    wvT = nc.dram_tensor("wvT", [D, PD], BF, kind="ExternalInput").ap()
    woT = nc.dram_tensor("woT", [PD, D], BF, kind="ExternalInput").ap()
    bq_d = nc.dram_tensor("bq", [PD, 1], F32, kind="ExternalInput").ap()
    bk_d = nc.dram_tensor("bk", [PD, 1], F32, kind="ExternalInput").ap()
    bv_d = nc.dram_tensor("bv", [PD, 1], F32, kind="ExternalInput").ap()
    cos_q = nc.dram_tensor("cos_q", [P, L], BF, kind="ExternalInput").ap()
    sin_q = nc.dram_tensor("sin_q", [P, L], BF, kind="ExternalInput").ap()
    cos_k = nc.dram_tensor("cos_k", [P, L], BF, kind="ExternalInput").ap()
    sin_k = nc.dram_tensor("sin_k", [P, L], BF, kind="ExternalInput").ap()
    outp = nc.dram_tensor("outp", [TOK, D], BF, kind="ExternalOutput").ap()

    with tile.TileContext(nc) as tc, ExitStack() as ctx:
        const = ctx.enter_context(tc.tile_pool(name="const", bufs=1))
        persist = ctx.enter_context(tc.tile_pool(name="persist", bufs=1))
        stage = ctx.enter_context(tc.tile_pool(name="stage", bufs=6))
        raws = ctx.enter_context(tc.tile_pool(name="raws", bufs=2))
        rots = ctx.enter_context(tc.tile_pool(name="rots", bufs=2))
        expp = ctx.enter_context(tc.tile_pool(name="expp", bufs=4))
        outs = ctx.enter_context(tc.tile_pool(name="outs", bufs=3))
        smalls = ctx.enter_context(tc.tile_pool(name="smalls", bufs=2))
        mmp = ctx.enter_context(tc.tile_pool(name="mmp", bufs=2, space="PSUM"))
        ctxp = ctx.enter_context(tc.tile_pool(name="ctxp", bufs=1, space="PSUM"))
        rsp = ctx.enter_context(tc.tile_pool(name="rsp", bufs=1, space="PSUM"))
        vhtp = ctx.enter_context(tc.tile_pool(name="vhtp", bufs=1))

        # ---- constants into SBUF (emitted in phase order so the first
        # projection's matmuls aren't queued behind 6MB of const DMA) ----
        def load_w(name, w_d):
            w_sb = const.tile([P, 8 * P], BF, name=name)
            nc.sync.dma_start(
                w_sb.rearrange("p (a m) -> p a m", a=8),
                w_d.rearrange("(a p) m -> p a m", p=P),
            )
            return w_sb

        def load_c(name, t_d, shape):
            t_sb = const.tile([P, shape], BF, name=name)
            nc.sync.dma_start(t_sb[:], t_d[:])
            return t_sb

        def load_b(name, b_d):
            b_sb = const.tile([P, 1], F32, name=name)
            nc.sync.dma_start(b_sb[:], b_d[:])
            return b_sb

        wq_sb = load_w("wq_sb", wqT)
        bq_sb = load_b("bq_sb", bq_d)
        cq_sb = load_c("cq_sb", cos_q, L)
        sq_sb = load_c("sq_sb", sin_q, L)

        # persistent activations
        qq_sb = persist.tile([P, TOK], BF)   # roped q-heads  [128 dims, 4096 tok]
        kk_sb = persist.tile([P, TOK], BF)   # roped k-heads
        vh_sb = persist.tile([P, TOK], BF)   # v-heads (dims-major)
        ctx_sb = persist.tile([P, TOK], BF)  # normalized attention ctx
        # Block-diagonal attention operands (both heads packed into K=128 so
        # the PE array runs fully occupied and the HAM clock-gate opens to
        # 2.4 GHz — K=64 matmul streams were measured to stay at 1.2 GHz):
        #   kh2[b]: 32 chunk tiles [128, 128]; chunk c is
        #           [[kh_h0[d, ktA] , 0], [0, kh_h1[d, ktA]]], ktA = 64 tokens
        #   vh2[b]: 32 chunk tiles [128, 128]; chunk c is
        #           [[vh_h0[ktA, d] , 0], [0, vh_h1[ktA, d]]]
        kh2 = [persist.tile([P, 32 * P], BF, name=f"kh2_{b}") for b in range(B)]
        vh2 = [persist.tile([P, 32 * P], BF, name=f"vh2_{b}") for b in range(B)]
        for t in kh2 + vh2:
            nc.gpsimd.memset(t[:], 0.0)
        # ones2: col 0 sums h0 rows (k 0..63), col 64 sums h1 rows; rest zero
        # (rowsums land on partitions 0 and 64 — legal AP base partitions).
        # Padded to a full 128-wide stationary so the rowsum matmuls keep the
        # PE array fully active.
        ones2 = const.tile([P, P], BF)
        nc.gpsimd.memset(ones2[:], 0.0)
        nc.vector.memset(ones2[0:DK, 0:1], 1.0)
        nc.vector.memset(ones2[DK:P, DK:DK + 1], 1.0)

        # ---------- phase helpers ----------
        def proj(x_d, w_sb, bias_sb, g, dst_sb, cos_sb=None, sin_sb=None):
            """Project token half g (2048 tokens) and optionally apply RoPE.

            Writes dst_sb[:, g*2048:(g+1)*2048] (bf16).
            """
            ps = [mmp.tile([P, 1024], F32, name=f"pj{g}_{half}", tag="mm")
                  for half in range(2)]
            for kt in range(8):
                xt = stage.tile([P, L], BF, name="xstage", tag="stage")
                nc.sync.dma_start(
                    xt[:], x_d[kt * P:(kt + 1) * P, g * L:(g + 1) * L])
                for half in range(2):
                    for nb in range(2):
                        c0 = half * 1024 + nb * 512
                        nc.tensor.matmul(
                            ps[half][:, nb * 512:(nb + 1) * 512],
                            lhsT=w_sb[:, kt * P:(kt + 1) * P],
                            rhs=xt[:, c0:c0 + 512],
                            start=(kt == 0), stop=(kt == 7),
                        )
            if cos_sb is None:
                # no rope (v): evict straight to destination
                for half in range(2):
                    nc.scalar.activation(
                        dst_sb[:, g * L + half * 1024: g * L + (half + 1) * 1024],
                        ps[half][:], AF.Identity, bias=bias_sb[:])
                return
            raw = raws.tile([P, L], BF, name="raw", tag="raw")
            for half in range(2):
                nc.scalar.activation(
                    raw[:, half * 1024:(half + 1) * 1024],
                    ps[half][:], AF.Identity, bias=bias_sb[:])
            rot = rots.tile([P, L], BF, name="rot", tag="rot")
            # rotate-half as partition-block moves (sign folded into sin table)
            for h in range(HPC):
                r0 = h * DK
                nc.sync.dma_start(rot[r0:r0 + 32, :], raw[r0 + 32:r0 + 64, :])
                nc.sync.dma_start(rot[r0 + 32:r0 + 64, :], raw[r0:r0 + 32, :])
            dst = dst_sb[:, g * L:(g + 1) * L]
            nc.vector.tensor_mul(raw[:], raw[:], cos_sb[:])
            nc.vector.tensor_mul(rot[:], rot[:], sin_sb[:])
            nc.vector.tensor_add(dst, raw[:], rot[:])

        def build_kv2(b):
            """Fill this batch's block-diagonal kh2/vh2 operand buffers."""
            # kh2: both copies are partition-aligned (h1 dims already live on
            # partitions 64..127 of kk_sb)
            kh2_r = kh2[b].rearrange("p (c u) -> p c u", u=P)
            kk_b = kk_sb[:, b * L:(b + 1) * L]
            nc.vector.tensor_copy(
                kh2_r[0:DK, :, 0:DK],
                kk_b[0:DK, :].rearrange("p (c u) -> p c u", u=DK))
            nc.vector.tensor_copy(
                kh2_r[DK:P, :, DK:P],
                kk_b[DK:P, :].rearrange("p (c u) -> p c u", u=DK))
            # vh2 needs [token, dim] tiles: PE-transpose 128-token tiles of
            # vh_sb, then 4 strided SBUF->SBUF DMAs place the 64-token
            # half-tiles into their diagonal blocks
            vht = vhtp.tile([P, 16 * P], BF, name="vht", tag="vht")
            for t in range(16):
                pt = mmp.tile([P, P], BF, name="pt", tag="mm")
                nc.tensor.transpose(
                    pt[:], vh_sb[:, b * L + t * P: b * L + (t + 1) * P],
                    ident[:])
                nc.vector.tensor_copy(vht[:, t * P:(t + 1) * P], pt[:])
            vht_r = vht.rearrange("p (t u) -> p t u", u=P)
            vh2_r = vh2[b].rearrange("p (t x) -> p t x", x=2 * P)
            # even chunks come from vht rows 0..63, odd chunks from 64..127
            nc.sync.dma_start(vh2_r[0:DK, :, 0:DK], vht_r[0:DK, :, 0:DK])
            nc.sync.dma_start(vh2_r[0:DK, :, 2 * DK:3 * DK], vht_r[DK:P, :, 0:DK])
            nc.sync.dma_start(vh2_r[DK:P, :, DK:2 * DK], vht_r[0:DK, :, DK:P])
            nc.sync.dma_start(vh2_r[DK:P, :, 3 * DK:4 * DK], vht_r[DK:P, :, DK:P])

        def attention(b, q2):
            """Both heads at once via block-diagonal K=128 matmuls.

            Scores chunk c: sc[0:64]  = scoresT_h0[ktA, qt],
                            sc[64:128] = scoresT_h1[ktA, qt].
            ctx2 accumulates [h0 dims | h1 dims, qt]; a parallel ones2 matmul
            accumulates both heads' softmax denominators in rows 0/1 of rs.
            Software-pipelined so the PE never waits on ScalarE's exp.
            """
            qs = qq_sb[:, b * L + q2 * 1024: b * L + (q2 + 1) * 1024]
            cp = ctxp.tile([P, 1024], F32, name="cp", tag="ctx")
            rs = rsp.tile([P, 1024], F32, name="rs", tag="rs")
            ex_prev = None
            for c in range(33):
                ex_cur = None
                if c < 32:
                    sc = mmp.tile([P, 1024], F32, name="sc", tag="mm")
                    for nb in range(2):
                        nc.tensor.matmul(
                            sc[:, nb * 512:(nb + 1) * 512],
                            lhsT=kh2[b][:, c * P:(c + 1) * P],
                            rhs=qs[:, nb * 512:(nb + 1) * 512],
                            start=True, stop=True, skip_group_check=True,
                        )
                    ex_cur = expp.tile([P, 1024], BF, name="ex", tag="exp")
                    nc.scalar.activation(ex_cur[:], sc[:], AF.Exp)
                    if debug_dumps and b == 0 and q2 == 0 and c == 0:
                        nc.sync.dma_start(dbg["dbg_exp"][:], ex_cur[:])
                if c >= 1:
                    cpv = c - 1
                    for nb in range(2):
                        sl = slice(nb * 512, (nb + 1) * 512)
                        nc.tensor.matmul(
                            cp[:, sl], lhsT=vh2[b][:, cpv * P:(cpv + 1) * P],
                            rhs=ex_prev[:, sl],
                            start=(cpv == 0), stop=(cpv == 31),
                            skip_group_check=True,
                        )
                        nc.tensor.matmul(
                            rs[:, sl], lhsT=ones2[:], rhs=ex_prev[:, sl],
                            start=(cpv == 0), stop=(cpv == 31),
                            skip_group_check=True,
                        )
                ex_prev = ex_cur
            # normalize: evict fast, then finish in SBUF off the PE path.
            # (base-64-partition custom DVE/GpSimd ops misbehave on HW, so
            # everything runs on partition-0-based tiles with small DMAs
            # doing the partition moves)
            rsum = smalls.tile([65, 1024], F32, name="rsum", tag="rsum")
            nc.vector.tensor_copy(rsum[:], rs[0:65, :])
            rsA = smalls.tile([1, 1024], F32, name="rsA", tag="rsA")
            rsB = smalls.tile([1, 1024], F32, name="rsB", tag="rsB")
            nc.sync.dma_start(rsA[:], rsum[0:1, :])
            nc.sync.dma_start(rsB[:], rsum[64:65, :])
            recA = smalls.tile([1, 1024], F32, name="recA", tag="recA")
            recB = smalls.tile([1, 1024], F32, name="recB", tag="recB")
            nc.vector.reciprocal_approx_fast(recA[:], rsA[:])
            nc.vector.reciprocal_approx_fast(recB[:], rsB[:])
            craw = smalls.tile([P, 1024], BF, name="craw", tag="craw")
            nc.vector.tensor_copy(craw[:], cp[:])
            bcs = smalls.tile([P, 1024], F32, name="bcs", tag="bcs")
            bcsB = smalls.tile([DK, 1024], F32, name="bcsB", tag="bcsB")
            nc.gpsimd.partition_broadcast(bcs[0:DK, :], recA[:], channels=DK)
            nc.gpsimd.partition_broadcast(bcsB[:], recB[:], channels=DK)
            nc.sync.dma_start(bcs[DK:P, :], bcsB[:])
            if debug_dumps and b == 0 and q2 == 0:
                cpd = smalls.tile([P, 1024], F32, name="cpd", tag="cpd", bufs=1)
                nc.vector.tensor_copy(cpd[:], cp[:])
                nc.sync.dma_start(dbg["dbg_cp"][:], cpd[:])
                nc.sync.dma_start(dbg["dbg_rec"][0:1, :], recA[:])
                nc.sync.dma_start(dbg["dbg_rec"][1:2, :], recB[:])
                nc.sync.dma_start(dbg["dbg_rsum"][0:1, :], rsum[0:1, :])
                nc.sync.dma_start(dbg["dbg_rsum"][1:2, :], rsum[64:65, :])
                nc.sync.dma_start(dbg["dbg_bcs"][:], bcs[:])
            c0 = b * L + q2 * 1024
            nc.vector.tensor_mul(ctx_sb[:, c0:c0 + 1024], craw[:], bcs[:])

        def out_proj(b, q2):
            for tb in range(q2 * 8, q2 * 8 + 8):
                t0 = b * L + tb * P
                po = mmp.tile([P, D], F32, name="po", tag="mm")
                for nb in range(2):
                    nc.tensor.matmul(
                        po[:, nb * 512:(nb + 1) * 512],
                        lhsT=ctx_sb[:, t0:t0 + P],
                        rhs=wo_sb[:, nb * 512:(nb + 1) * 512],
                        start=True, stop=True, skip_group_check=True,
                    )
                ob = outs.tile([P, D], BF, name="ob", tag="out")
                nc.vector.tensor_copy(ob[:], po[:])
                nc.sync.dma_start(outp[t0:t0 + P, :], ob[:])

        # ---------- program ----------
        proj(qT, wq_sb, bq_sb, 0, qq_sb, cq_sb, sq_sb)
        wk_sb = load_w("wk_sb", wkT)
        bk_sb = load_b("bk_sb", bk_d)
        ck_sb = load_c("ck_sb", cos_k, L)
        sk_sb = load_c("sk_sb", sin_k, L)
        proj(kT, wk_sb, bk_sb, 0, kk_sb, ck_sb, sk_sb)
        wv_sb = load_w("wv_sb", wvT)
        bv_sb = load_b("bv_sb", bv_d)
        ident = const.tile([P, P], BF)
        from concourse.masks import make_identity
        make_identity(nc, ident[:])
        wo_sb = const.tile([P, D], BF)
        nc.sync.dma_start(wo_sb[:], woT[:])
        proj(vT, wv_sb, bv_sb, 0, vh_sb)
        build_kv2(0)
        attention(0, 0)
        attention(0, 1)
        out_proj(0, 0)
        out_proj(0, 1)
        proj(qT, wq_sb, bq_sb, 1, qq_sb, cq_sb, sq_sb)
        proj(kT, wk_sb, bk_sb, 1, kk_sb, ck_sb, sk_sb)
        proj(vT, wv_sb, bv_sb, 1, vh_sb)
        build_kv2(1)
        attention(1, 0)
        out_proj(1, 0)
        attention(1, 1)
        out_proj(1, 1)

        if debug_dumps:
            nc.sync.dma_start(dbg["dbg_qq"][:], qq_sb[:])
            nc.sync.dma_start(dbg["dbg_kk"][:], kk_sb[:])
            nc.sync.dma_start(dbg["dbg_vh"][:], vh_sb[:])
            nc.sync.dma_start(dbg["dbg_ctx"][:], ctx_sb[:])
            nc.sync.dma_start(dbg["dbg_vaug"][:, 0:32 * P], kh2[0][:])
            nc.sync.dma_start(dbg["dbg_vaug"][:, 32 * P:64 * P], vh2[0][:])

    return nc


def _rope_tables():
    """Host-built RoPE tables, transposed to [d, t], 2 heads stacked.

    sin is sign-folded for the rotate-half convention; q tables carry the
    1/sqrt(dk) attention scale.
    """
    inv_freq = 1.0 / (ROPE_BASE ** (np.arange(0, DK, 2, dtype=np.float64) / DK))
    t = np.arange(L, dtype=np.float64)
    ang = np.outer(t, inv_freq)               # [L, 32]
    emb = np.concatenate([ang, ang], axis=1)  # [L, 64]
    cos = np.cos(emb).T.astype(np.float32)    # [64, L]
    sin = np.sin(emb).T.astype(np.float32)
    sin_folded = sin.copy()
    sin_folded[:32] *= -1.0
    scale = 1.0 / np.sqrt(DK)
    cos2 = np.concatenate([cos, cos], axis=0)                # [128, L]
    sin2 = np.concatenate([sin_folded, sin_folded], axis=0)  # [128, L]
    bf = ml_dtypes.bfloat16
    return (
        (cos2 * scale).astype(bf), (sin2 * scale).astype(bf),
        cos2.astype(bf), sin2.astype(bf),
    )


_NC_CACHE = {}


def _get_nc():
    if "nc" not in _NC_CACHE:
        nc = build_nc()
        nc.finalize()
        _NC_CACHE["nc"] = nc
    return _NC_CACHE["nc"]


def kernel(q, k, v, Wq, bq, Wk, bk, Wv, bv, Wo, bo):
    assert q.shape == (B, L, D) and k.shape == (B, L, D) and v.shape == (B, L, D)
    bf = ml_dtypes.bfloat16
    qT = np.ascontiguousarray(q.reshape(TOK, D).T).astype(bf)
    kT = np.ascontiguousarray(k.reshape(TOK, D).T).astype(bf)
    vT = np.ascontiguousarray(v.reshape(TOK, D).T).astype(bf)
    cos_q, sin_q, cos_k, sin_k = _rope_tables()

    in_maps = []
    for c in range(NCORES):
        hs = slice(c * PD, (c + 1) * PD)
        in_maps.append({
            "qT": qT, "kT": kT, "vT": vT,
            "wqT": np.ascontiguousarray(Wq[hs, :].T).astype(bf),
            "wkT": np.ascontiguousarray(Wk[hs, :].T).astype(bf),
            "wvT": np.ascontiguousarray(Wv[hs, :].T).astype(bf),
            "woT": np.ascontiguousarray(Wo[:, hs].T).astype(bf),
            "bq": np.asarray(bq[hs], np.float32).reshape(PD, 1),
            "bk": np.asarray(bk[hs], np.float32).reshape(PD, 1),
            "bv": np.asarray(bv[hs], np.float32).reshape(PD, 1),
            "cos_q": cos_q, "sin_q": sin_q, "cos_k": cos_k, "sin_k": sin_k,
        })

    nc = _get_nc()
    res = run_bass_kernel_spmd(nc, in_maps, list(range(NCORES)))
    out = np.zeros((TOK, D), np.float64)
    for r in res.results:
        out += r["outp"].astype(np.float64)
    out += np.asarray(bo, np.float64)[None, :]
    return out.astype(np.float32).reshape(B, L, D)



# revision 9
# speedup vs baseline: 1.3802x; 1.3802x over previous
"""Trainium2 Bass kernel for MultiHeadAttention with RoPE.

Problem: B=2, L=2048, d_model=1024, 16 heads, d_k=64, fp32 in/out.

Sharding (8 cores): batch x head-group.  Core c owns batch c//4 and the 4
heads 4*(c%4)..4*(c%4)+3 (a 256-wide slice of the projection dims).  Each
core reads only its batch's q/k/v (transposed + bf16 on host), its 256-row
slice of Wq/Wk/Wv (pre-transposed) and the matching 256 columns of Wo.
The host sums the 4 partial outputs per batch and adds bo.

Per-core pipeline (all matmuls bf16, fp32 PSUM accumulation):
  1. QKV projections: [256 dims, 2048 tok], K=1024 in 8 chunks
  2. RoPE via partition-shifted DMA copy + 3 DVE ops; 1/sqrt(dk) and the
     rotate-half sign are folded into host-built cos/sin tables
  3. scores per head via SAME-HEAD block-diagonal packing: chunk c is
     diag(kh[:, 128c:128c+64], kh[:, 128c+64:128c+128]) against a
     partition-duplicated q rhs -> PSUM [128 contiguous kt, qt]
  4. exp on ScalarE (no max-subtract: scores ~ N(0,1)), bf16 out
  5. ctx per head: dense K=128 kt chunks, stationary vaug [128 kt, 65]
     whose 65th column is ones -> row 64 of the PSUM accumulator is the
     softmax denominator for free
  6. normalize: reciprocal + gpsimd partition broadcast + DVE multiply
  7. out_partial[tok, 1024] = ctx (stationary, K=256 in 2 passes) @ WoT
"""

import numpy as np
import ml_dtypes

import concourse.bass as bass
import concourse.mybir as mybir
import concourse.tile as tile
from concourse import bacc
from concourse.bass_utils import run_bass_kernel_spmd

BF = mybir.dt.bfloat16
F32 = mybir.dt.float32
AF = mybir.ActivationFunctionType

NCORES = 8
B = 2
L = 2048
D = 1024          # d_model
H = 16            # heads
DK = 64           # head dim
HPC = 4           # heads per core
PD = HPC * DK     # projection dims per core = 256
TOK = L           # tokens per core (one batch)
P = 128
NMT = PD // P     # matmul M-tiles per projection = 2

ROPE_BASE = 10000.0


def build_nc(debug_dumps=False):
    """Build the single-core Bass program (SPMD: same program, per-core data)."""
    from contextlib import ExitStack

    nc = bacc.Bacc("TRN2", target_bir_lowering=False, debug=False)
    dbg = {}
    if debug_dumps:
        for nm, shp, dt in [
            ("dbg_vaug", [P, 16 * 65], BF), ("dbg_qs2", [P, L], BF),
            ("dbg_kh2", [P, L], BF), ("dbg_vh", [P, L], BF),
            ("dbg_ex", [P, 1024], BF), ("dbg_cfull", [65, 1024], F32),
            ("dbg_rec", [1, 1024], F32), ("dbg_bcs", [DK, 1024], F32),
            ("dbg_ctx", [P, L], BF),
        ]:
            dbg[nm] = nc.dram_tensor(nm, shp, dt, kind="ExternalOutput").ap()

    # ---- DRAM I/O ----
    qT = nc.dram_tensor("qT", [D, TOK], BF, kind="ExternalInput").ap()
    kT = nc.dram_tensor("kT", [D, TOK], BF, kind="ExternalInput").ap()
    vT = nc.dram_tensor("vT", [D, TOK], BF, kind="ExternalInput").ap()
    wqT = nc.dram_tensor("wqT", [D, PD], BF, kind="ExternalInput").ap()
    wkT = nc.dram_tensor("wkT", [D, PD], BF, kind="ExternalInput").ap()
    wvT = nc.dram_tensor("wvT", [D, PD], BF, kind="ExternalInput").ap()
    woT = nc.dram_tensor("woT", [PD, D], BF, kind="ExternalInput").ap()
    bq_d = nc.dram_tensor("bq", [PD, 1], F32, kind="ExternalInput").ap()
    bk_d = nc.dram_tensor("bk", [PD, 1], F32, kind="ExternalInput").ap()
    bv_d = nc.dram_tensor("bv", [PD, 1], F32, kind="ExternalInput").ap()
    cos_q = nc.dram_tensor("cos_q", [P, L], BF, kind="ExternalInput").ap()
    sin_q = nc.dram_tensor("sin_q", [P, L], BF, kind="ExternalInput").ap()
    cos_k = nc.dram_tensor("cos_k", [P, L], BF, kind="ExternalInput").ap()
    sin_k = nc.dram_tensor("sin_k", [P, L], BF, kind="ExternalInput").ap()
    outp = nc.dram_tensor("outp", [TOK, D], BF, kind="ExternalOutput").ap()

    with tile.TileContext(nc) as tc, ExitStack() as ctx:
        const = ctx.enter_context(tc.tile_pool(name="const", bufs=1))
        persist = ctx.enter_context(tc.tile_pool(name="persist", bufs=1))
        stage = ctx.enter_context(tc.tile_pool(name="stage", bufs=8))
        raws = ctx.enter_context(tc.tile_pool(name="raws", bufs=2))
        rots = ctx.enter_context(tc.tile_pool(name="rots", bufs=2))
        expp = ctx.enter_context(tc.tile_pool(name="expp", bufs=6))
        outs = ctx.enter_context(tc.tile_pool(name="outs", bufs=3))
        smalls = ctx.enter_context(tc.tile_pool(name="smalls", bufs=2))
        mmp = ctx.enter_context(tc.tile_pool(name="mmp", bufs=2, space="PSUM"))
        cpp = ctx.enter_context(tc.tile_pool(name="cpp", bufs=2, space="PSUM"))

        # ---- constants into SBUF (phase order: q-proj consts first) ----
        def load_w(name, w_d):
            # [1024, 256] -> [128, 8, 256]
            w_sb = const.tile([P, 8 * PD], BF, name=name)
            nc.sync.dma_start(
                w_sb.rearrange("p (a m) -> p a m", a=8),
                w_d.rearrange("(a p) m -> p a m", p=P),
            )
            return w_sb

        def load_b(name, b_d):
            # [256, 1] -> [128, 2]
            b_sb = const.tile([P, NMT], F32, name=name)
            nc.sync.dma_start(
                b_sb.rearrange("p (a m) -> p a m", a=NMT),
                b_d.rearrange("(a p) m -> p a m", p=P),
            )
            return b_sb

        def load_c(name, t_d):
            t_sb = const.tile([P, L], BF, name=name)
            nc.sync.dma_start(t_sb[:], t_d[:])
            return t_sb

        wq_sb = load_w("wq_sb", wqT)
        bq_sb = load_b("bq_sb", bq_d)
        cq_sb = load_c("cq_sb", cos_q)
        sq_sb = load_c("sq_sb", sin_q)

        # persistent per-head attention operands
        # qs2[h]: [qh_h; qh_h] partition-duplicated, [128, 2048]
        # kh2[h]: same-head block-diag chunks, [128, 16*128]
        # vaug[h]: 16 chunks of [128 kt, 65] (col 64 = ones), [128, 1040]
        qs2 = [persist.tile([P, L], BF, name=f"qs2_{h}") for h in range(HPC)]
        kh2 = [persist.tile([P, L], BF, name=f"kh2_{h}") for h in range(HPC)]
        vaug = [persist.tile([P, 16 * 65], BF, name=f"vaug_{h}")
                for h in range(HPC)]
        vh_sb = [persist.tile([P, L], BF, name=f"vh_{m}") for m in range(NMT)]
        ctx_sb = [persist.tile([P, L], BF, name=f"ctx_{m}") for m in range(NMT)]
        for t in kh2:
            nc.gpsimd.memset(t[:], 0.0)
        for t in vaug:
            nc.vector.memset(
                t.rearrange("p (c u) -> p c u", u=65)[:, :, 64:65], 1.0)

        # ---------- phase helpers ----------
        def proj(x_d, w_sb, bias_sb, dst_of_mt, cos_sb=None, sin_sb=None,
                 post=None):
            """Project x [1024, 2048] -> [256, 2048]; optional RoPE.

            dst_of_mt(mt) gives the bf16 destination [128, 2048] for raw
            (v) output; with RoPE the roped result goes through `post(mt,
            rot_tile)` to build qs2/kh2.
            """
            xs = [stage.tile([P, L], BF, name="xstage", tag="stage")
                  for _ in range(8)]
            for kc in range(8):
                nc.sync.dma_start(xs[kc][:], x_d[kc * P:(kc + 1) * P, :])
            w_r = w_sb.rearrange("p (a m) -> p a m", a=8)
            for mt in range(NMT):
                ps = [mmp.tile([P, 1024], F32, name=f"pj{mt}{j}", tag="mm")
                      for j in range(2)]
                for kc in range(8):
                    for j in range(2):
                        for nb in range(2):
                            c0 = j * 1024 + nb * 512
                            nc.tensor.matmul(
                                ps[j][:, nb * 512:(nb + 1) * 512],
                                lhsT=w_r[:, kc, mt * P:(mt + 1) * P],
                                rhs=xs[kc][:, c0:c0 + 512],
                                start=(kc == 0), stop=(kc == 7),
                                skip_group_check=True,
                            )
                if cos_sb is None:
                    dst = dst_of_mt(mt)
                    for j in range(2):
                        nc.scalar.activation(
                            dst[:, j * 1024:(j + 1) * 1024], ps[j][:],
                            AF.Identity, bias=bias_sb[:, mt:mt + 1])
                    continue
                raw = raws.tile([P, L], BF, name="raw", tag="raw")
                for j in range(2):
                    nc.scalar.activation(
                        raw[:, j * 1024:(j + 1) * 1024], ps[j][:],
                        AF.Identity, bias=bias_sb[:, mt:mt + 1])
                rot = rots.tile([P, L], BF, name="rot", tag="rot")
                # rotate-half as partition-block moves (sign folded into sin)
                for hb in range(2):
                    r0 = hb * DK
                    nc.sync.dma_start(rot[r0:r0 + 32, :], raw[r0 + 32:r0 + 64, :])
                    nc.sync.dma_start(rot[r0 + 32:r0 + 64, :], raw[r0:r0 + 32, :])
                nc.vector.tensor_mul(raw[:], raw[:], cos_sb[:])
                nc.vector.tensor_mul(rot[:], rot[:], sin_sb[:])
                nc.vector.tensor_add(rot[:], rot[:], raw[:])
                post(mt, rot)

        def build_q(mt, rope_t):
            """qs2 for heads 2mt, 2mt+1: partition-duplicate each head."""
            for hl in range(2):
                h = 2 * mt + hl
                src = rope_t[hl * DK:(hl + 1) * DK, :]
                nc.vector.tensor_copy(qs2[h][0:DK, :], src)
                nc.vector.tensor_copy(qs2[h][DK:P, :], src)

        def build_k(mt, rope_t):
            """kh2 for heads 2mt, 2mt+1: same-head block-diagonal chunks."""
            for hl in range(2):
                h = 2 * mt + hl
                src = rope_t[hl * DK:(hl + 1) * DK, :].rearrange(
                    "p (c g u) -> p c g u", g=2, u=DK)
                dst = kh2[h].rearrange("p (c g u) -> p c g u", g=2, u=DK)
                nc.vector.tensor_copy(dst[0:DK, :, 0, :], src[:, :, 0, :])
                nc.vector.tensor_copy(dst[DK:P, :, 1, :], src[:, :, 1, :])

        def build_vaug(mt):
            """Transpose vh [dims, kt] -> vaug [kt, dims] via PE transpose."""
            dsts = [vaug[2 * mt + hl].rearrange("p (c u) -> p c u", u=65)
                    for hl in range(2)]
            for c in range(16):
                pt = mmp.tile([P, P], BF, name="pt", tag="mm")
                nc.tensor.transpose(
                    pt[:], vh_sb[mt][:, c * P:(c + 1) * P], ident[:])
                for hl in range(2):
                    nc.vector.tensor_copy(
                        dsts[hl][:, c, 0:DK], pt[:, hl * DK:(hl + 1) * DK])

        def attention(h, j):
            """Head h, query block j (1024 tokens): scores/exp/ctx."""
            qs = qs2[h][:, j * 1024:(j + 1) * 1024]
            cp = cpp.tile([65, 1024], F32, name="cp", tag="cp")
            ex_prev = None
            for c in range(17):
                ex_cur = None
                if c < 16:
                    sc = mmp.tile([P, 1024], F32, name="sc", tag="mm")
                    for nb in range(2):
                        nc.tensor.matmul(
                            sc[:, nb * 512:(nb + 1) * 512],
                            lhsT=kh2[h][:, c * P:(c + 1) * P],
                            rhs=qs[:, nb * 512:(nb + 1) * 512],
                            start=True, stop=True, skip_group_check=True,
                        )
                    ex_cur = expp.tile([P, 1024], BF, name="ex", tag="exp")
                    nc.scalar.activation(ex_cur[:], sc[:], AF.Exp)
                    if debug_dumps and h == 0 and j == 0 and c == 0:
                        nc.sync.dma_start(dbg["dbg_ex"][:], ex_cur[:])
                if c >= 1:
                    cpv = c - 1
                    for nb in range(2):
                        sl = slice(nb * 512, (nb + 1) * 512)
                        nc.tensor.matmul(
                            cp[:, sl],
                            lhsT=vaug[h][:, cpv * 65:(cpv + 1) * 65],
                            rhs=ex_prev[:, sl],
                            start=(cpv == 0), stop=(cpv == 15),
                            skip_group_check=True,
                        )
                ex_prev = ex_cur
            # normalize: ctx_sb rows = craw * broadcast(1/den)
            cfull = smalls.tile([65, 1024], F32, name="cfull", tag="cfull")
            nc.vector.tensor_copy(cfull[:], cp[:])
            den = smalls.tile([1, 1024], F32, name="den", tag="den")
            nc.sync.dma_start(den[:], cfull[64:65, :])
            rec = smalls.tile([1, 1024], F32, name="rec", tag="rec")
            nc.vector.reciprocal_approx_fast(rec[:], den[:])
            bcs = smalls.tile([DK, 1024], F32, name="bcs", tag="bcs")
            nc.gpsimd.partition_broadcast(bcs[:], rec[:], channels=DK)
            if debug_dumps and h == 0 and j == 0:
                nc.sync.dma_start(dbg["dbg_cfull"][:], cfull[:])
                nc.sync.dma_start(dbg["dbg_rec"][:], rec[:])
                nc.sync.dma_start(dbg["dbg_bcs"][:], bcs[:])
            mt, hl = h // 2, h % 2
            nc.vector.tensor_mul(
                ctx_sb[mt][hl * DK:(hl + 1) * DK, j * 1024:(j + 1) * 1024],
                cfull[0:DK, :], bcs[:])

        def out_proj(tb):
            """Token tile tb: [128, 1024] partial output, K=256 in 2 passes."""
            t0 = tb * P
            po = mmp.tile([P, D], F32, name="po", tag="mm")
            for mt in range(NMT):
                for nb in range(2):
                    nc.tensor.matmul(
                        po[:, nb * 512:(nb + 1) * 512],
                        lhsT=ctx_sb[mt][:, t0:t0 + P],
                        rhs=wo_sb[mt][:, nb * 512:(nb + 1) * 512],
                        start=(mt == 0), stop=(mt == NMT - 1),
                        skip_group_check=True,
                    )
            ob = outs.tile([P, D], BF, name="ob", tag="out")
            if tb % 2 == 0:
                nc.vector.tensor_copy(ob[:], po[:])
            else:
                nc.scalar.activation(ob[:], po[:], AF.Identity)
            nc.sync.dma_start(outp[t0:t0 + P, :], ob[:])

        # ---------- program ----------
        proj(qT, wq_sb, bq_sb, None, cq_sb, sq_sb, build_q)
        wk_sb = load_w("wk_sb", wkT)
        bk_sb = load_b("bk_sb", bk_d)
        ck_sb = load_c("ck_sb", cos_k)
        sk_sb = load_c("sk_sb", sin_k)
        proj(kT, wk_sb, bk_sb, None, ck_sb, sk_sb, build_k)
        wv_sb = load_w("wv_sb", wvT)
        bv_sb = load_b("bv_sb", bv_d)
        ident = const.tile([P, P], BF)
        from concourse.masks import make_identity
        make_identity(nc, ident[:])
        wo_sb = [const.tile([P, D], BF, name=f"wo_{m}") for m in range(NMT)]
        for m in range(NMT):
            nc.sync.dma_start(wo_sb[m][:], woT[m * P:(m + 1) * P, :])
        proj(vT, wv_sb, bv_sb, lambda mt: vh_sb[mt])
        build_vaug(0)
        build_vaug(1)
        for h in range(HPC):
            attention(h, 0)
            attention(h, 1)
        for tb in range(16):
            out_proj(tb)

        if debug_dumps:
            nc.sync.dma_start(dbg["dbg_vaug"][:], vaug[0][:])
            nc.sync.dma_start(dbg["dbg_qs2"][:], qs2[0][:])
            nc.sync.dma_start(dbg["dbg_kh2"][:], kh2[0][:])
            nc.sync.dma_start(dbg["dbg_vh"][:], vh_sb[0][:])
            nc.sync.dma_start(dbg["dbg_ctx"][:], ctx_sb[0][:])

    return nc


def _rope_tables():
    """Host-built RoPE tables, transposed to [d, t], 2 heads stacked.

    sin is sign-folded for the rotate-half convention; q tables carry the
    1/sqrt(dk) attention scale.
    """
    inv_freq = 1.0 / (ROPE_BASE ** (np.arange(0, DK, 2, dtype=np.float64) / DK))
    t = np.arange(L, dtype=np.float64)
    ang = np.outer(t, inv_freq)               # [L, 32]
    emb = np.concatenate([ang, ang], axis=1)  # [L, 64]
    cos = np.cos(emb).T.astype(np.float32)    # [64, L]
    sin = np.sin(emb).T.astype(np.float32)
    sin_folded = sin.copy()
    sin_folded[:32] *= -1.0
    scale = 1.0 / np.sqrt(DK)
    cos2 = np.concatenate([cos, cos], axis=0)                # [128, L]
    sin2 = np.concatenate([sin_folded, sin_folded], axis=0)  # [128, L]
    bf = ml_dtypes.bfloat16
    return (
        (cos2 * scale).astype(bf), (sin2 * scale).astype(bf),
        cos2.astype(bf), sin2.astype(bf),
    )


_NC_CACHE = {}


def _get_nc():
    if "nc" not in _NC_CACHE:
        nc = build_nc()
        nc.finalize()
        _NC_CACHE["nc"] = nc
    return _NC_CACHE["nc"]


def make_in_maps(q, k, v, Wq, bq, Wk, bk, Wv, bv, Wo, bo):
    bf = ml_dtypes.bfloat16
    cos_q, sin_q, cos_k, sin_k = _rope_tables()
    xT = {}
    for b in range(B):
        xT[("q", b)] = np.ascontiguousarray(np.asarray(q)[b].T).astype(bf)
        xT[("k", b)] = np.ascontiguousarray(np.asarray(k)[b].T).astype(bf)
        xT[("v", b)] = np.ascontiguousarray(np.asarray(v)[b].T).astype(bf)
    in_maps = []
    for c in range(NCORES):
        b, g = c // 4, c % 4
        hs = slice(g * PD, (g + 1) * PD)
        in_maps.append({
            "qT": xT[("q", b)], "kT": xT[("k", b)], "vT": xT[("v", b)],
            "wqT": np.ascontiguousarray(np.asarray(Wq)[hs, :].T).astype(bf),
            "wkT": np.ascontiguousarray(np.asarray(Wk)[hs, :].T).astype(bf),
            "wvT": np.ascontiguousarray(np.asarray(Wv)[hs, :].T).astype(bf),
            "woT": np.ascontiguousarray(np.asarray(Wo)[:, hs].T).astype(bf),
            "bq": np.asarray(bq[hs], np.float32).reshape(PD, 1),
            "bk": np.asarray(bk[hs], np.float32).reshape(PD, 1),
            "bv": np.asarray(bv[hs], np.float32).reshape(PD, 1),
            "cos_q": cos_q, "sin_q": sin_q, "cos_k": cos_k, "sin_k": sin_k,
        })
    return in_maps


def kernel(q, k, v, Wq, bq, Wk, bk, Wv, bv, Wo, bo):
    assert q.shape == (B, L, D) and k.shape == (B, L, D) and v.shape == (B, L, D)
    in_maps = make_in_maps(q, k, v, Wq, bq, Wk, bk, Wv, bv, Wo, bo)
    nc = _get_nc()
    res = run_bass_kernel_spmd(nc, in_maps, list(range(NCORES)))
    out = np.zeros((B, TOK, D), np.float64)
    for c, r in enumerate(res.results):
        out[c // 4] += r["outp"].astype(np.float64)
    out += np.asarray(bo, np.float64)[None, None, :]
    return out.astype(np.float32)


# revision 13
# speedup vs baseline: 1.4515x; 1.0517x over previous
"""Trainium2 Bass kernel for MultiHeadAttention with RoPE.

Problem: B=2, L=2048, d_model=1024, 16 heads, d_k=64, fp32 in/out.

Sharding (8 cores): batch x head-group.  Core c owns batch c//4 and the 4
heads 4*(c%4)..4*(c%4)+3 (a 256-wide slice of the projection dims).  Each
core reads only its batch's q/k/v (transposed + bf16 on host), its 256-row
slice of Wq/Wk/Wv (pre-transposed) and the matching 256 columns of Wo.
The host sums the 4 partial outputs per batch and adds bo.

Per-core pipeline (all matmuls bf16, fp32 PSUM accumulation):
  1. QKV projections: [256 dims, 2048 tok], K=1024 in 8 chunks
  2. RoPE via partition-shifted DMA copy + 3 DVE ops; 1/sqrt(dk) and the
     rotate-half sign are folded into host-built cos/sin tables
  3. scores per head via SAME-HEAD block-diagonal packing: chunk c is
     diag(kh[:, 128c:128c+64], kh[:, 128c+64:128c+128]) against a
     partition-duplicated q rhs -> PSUM [128 contiguous kt, qt]
  4. exp on ScalarE (no max-subtract: scores ~ N(0,1)), bf16 out
  5. ctx per head: dense K=128 kt chunks, stationary vaug [128 kt, 65]
     whose 65th column is ones -> row 64 of the PSUM accumulator is the
     softmax denominator for free
  6. normalize: reciprocal + gpsimd partition broadcast + DVE multiply
  7. out_partial[tok, 1024] = ctx (stationary, K=256 in 2 passes) @ WoT

Scheduling: the exp stream on ScalarE is the pacing engine during
attention (~17.8us per head-block vs ~13.6us of PE work), so the second
head-pair's projections, the vaug transposes and the first 8 out-proj
tiles are emitted as fine-grained "fill" steps interleaved into the
attention chunk loops, keeping TensorE dense (no >3.4us idle gaps that
would re-throttle the HAM clock gate).
"""

import numpy as np
import ml_dtypes

import concourse.bass as bass
import concourse.mybir as mybir
import concourse.tile as tile
from concourse import bacc
from concourse.bass_utils import run_bass_kernel_spmd

BF = mybir.dt.bfloat16
F32 = mybir.dt.float32
AF = mybir.ActivationFunctionType
ALU = mybir.AluOpType

NCORES = 8
B = 2
L = 2048
D = 1024          # d_model
H = 16            # heads
DK = 64           # head dim
HPC = 4           # heads per core
PD = HPC * DK     # projection dims per core = 256
TOK = L           # tokens per core (one batch)
P = 128
NMT = PD // P     # matmul M-tiles per projection = 2

ROPE_BASE = 10000.0


def build_nc(debug_dumps=False):
    """Build the single-core Bass program (SPMD: same program, per-core data)."""
    from contextlib import ExitStack

    nc = bacc.Bacc("TRN2", target_bir_lowering=False, debug=False)
    dbg = {}
    if debug_dumps:
        for nm, shp, dt in [
            ("dbg_vaug", [P, 16 * 65], BF), ("dbg_qs2", [P, L], BF),
            ("dbg_kh2", [P, L], BF), ("dbg_vh", [P, L], BF),
            ("dbg_ex", [P, 1024], BF), ("dbg_cfull", [65, 1024], F32),
            ("dbg_rec", [1, 1024], F32), ("dbg_bcs", [DK, 1024], F32),
            ("dbg_ctx", [P, L], BF),
        ]:
            dbg[nm] = nc.dram_tensor(nm, shp, dt, kind="ExternalOutput").ap()

    # ---- DRAM I/O ----
    qT = nc.dram_tensor("qT", [D, TOK], BF, kind="ExternalInput").ap()
    kT = nc.dram_tensor("kT", [D, TOK], BF, kind="ExternalInput").ap()
    vT = nc.dram_tensor("vT", [D, TOK], BF, kind="ExternalInput").ap()
    wqT = nc.dram_tensor("wqT", [D, PD], BF, kind="ExternalInput").ap()
    wkT = nc.dram_tensor("wkT", [D, PD], BF, kind="ExternalInput").ap()
    wvT = nc.dram_tensor("wvT", [D, PD], BF, kind="ExternalInput").ap()
    woT = nc.dram_tensor("woT", [PD, D], BF, kind="ExternalInput").ap()
    bq_d = nc.dram_tensor("bq", [PD, 1], F32, kind="ExternalInput").ap()
    bk_d = nc.dram_tensor("bk", [PD, 1], F32, kind="ExternalInput").ap()
    bv_d = nc.dram_tensor("bv", [PD, 1], F32, kind="ExternalInput").ap()
    cos_q = nc.dram_tensor("cos_q", [P, L], BF, kind="ExternalInput").ap()
    sin_q = nc.dram_tensor("sin_q", [P, L], BF, kind="ExternalInput").ap()
    cos_k = nc.dram_tensor("cos_k", [P, L], BF, kind="ExternalInput").ap()
    sin_k = nc.dram_tensor("sin_k", [P, L], BF, kind="ExternalInput").ap()
    outp = nc.dram_tensor("outp", [TOK, D], BF, kind="ExternalOutput").ap()

    with tile.TileContext(nc) as tc, ExitStack() as ctx:
        const = ctx.enter_context(tc.tile_pool(name="const", bufs=1))
        persist = ctx.enter_context(tc.tile_pool(name="persist", bufs=1))
        stage = ctx.enter_context(tc.tile_pool(name="stage", bufs=9))
        raws = ctx.enter_context(tc.tile_pool(name="raws", bufs=2))
        rots = ctx.enter_context(tc.tile_pool(name="rots", bufs=2))
        expp = ctx.enter_context(tc.tile_pool(name="expp", bufs=5))
        outs = ctx.enter_context(tc.tile_pool(name="outs", bufs=3))
        smalls = ctx.enter_context(tc.tile_pool(name="smalls", bufs=2))
        scp = ctx.enter_context(tc.tile_pool(name="scp", bufs=2, space="PSUM"))
        fillp = ctx.enter_context(tc.tile_pool(name="fillp", bufs=1, space="PSUM"))
        cpp = ctx.enter_context(tc.tile_pool(name="cpp", bufs=1, space="PSUM"))

        # ---- constants into SBUF (phase order: q-proj consts first) ----
        def load_w(name, w_d):
            # [1024, 256] -> [128, 8, 256]
            w_sb = const.tile([P, 8 * PD], BF, name=name)
            nc.sync.dma_start(
                w_sb.rearrange("p (a m) -> p a m", a=8),
                w_d.rearrange("(a p) m -> p a m", p=P),
            )
            return w_sb

        def load_b(name, b_d):
            # [256, 1] -> [128, 2]
            b_sb = const.tile([P, NMT], F32, name=name)
            nc.sync.dma_start(
                b_sb.rearrange("p (a m) -> p a m", a=NMT),
                b_d.rearrange("(a p) m -> p a m", p=P),
            )
            return b_sb

        def load_c(name, t_d):
            t_sb = const.tile([P, L], BF, name=name)
            nc.sync.dma_start(t_sb[:], t_d[:])
            return t_sb

        wq_sb = load_w("wq_sb", wqT)
        bq_sb = load_b("bq_sb", bq_d)
        cq_sb = load_c("cq_sb", cos_q)
        sq_sb = load_c("sq_sb", sin_q)

        # persistent per-head attention operands
        qs2 = [persist.tile([P, L], BF, name=f"qs2_{h}") for h in range(HPC)]
        kh2 = [persist.tile([P, L], BF, name=f"kh2_{h}") for h in range(HPC)]
        vaug = [persist.tile([P, 16 * 65], BF, name=f"vaug_{h}")
                for h in range(HPC)]
        vh_sb = [persist.tile([P, L], BF, name=f"vh_{m}") for m in range(NMT)]
        ctx_sb = [persist.tile([P, L], BF, name=f"ctx_{m}") for m in range(NMT)]
        for t in kh2:
            nc.gpsimd.memset(t[:], 0.0)
        for t in vaug:
            nc.vector.memset(
                t.rearrange("p (c u) -> p c u", u=65)[:, :, 64:65], 1.0)

        wk_sb = load_w("wk_sb", wkT)
        bk_sb = load_b("bk_sb", bk_d)
        ck_sb = load_c("ck_sb", cos_k)
        sk_sb = load_c("sk_sb", sin_k)
        wv_sb = load_w("wv_sb", wvT)
        bv_sb = load_b("bv_sb", bv_d)
        ident = const.tile([P, P], BF)
        from concourse.masks import make_identity
        make_identity(nc, ident[:])
        wo_sb = [const.tile([P, D], BF, name=f"wo_{m}") for m in range(NMT)]
        for m in range(NMT):
            nc.sync.dma_start(wo_sb[m][:], woT[m * P:(m + 1) * P, :])
        ones_sb = const.tile([P, 1024], BF)
        nc.vector.memset(ones_sb[:], 1.0)

        # ---------- emission-step generators ----------
        def gen_proj(x_d, w_sb, bias_sb, mt, kind, on_act):
            """Yield fine-grained steps projecting x -> M-tile mt.

            kind: 'q' | 'k' | 'v'.  on_act: evict on ScalarE (pre-phase)
            vs DVE scalar_tensor_tensor (mid-attention fill).  All tile
            allocations happen inside the yielded closures so pool slot
            rotation follows EMISSION order, not generator-build order.
            """
            st = {}
            w_r = w_sb.rearrange("p (a m) -> p a m", a=8)

            def dma_all():
                st["xs"] = [stage.tile([P, L], BF, name="xstage", tag="stage")
                            for _ in range(8)]
                for kc in range(8):
                    nc.sync.dma_start(st["xs"][kc][:],
                                      x_d[kc * P:(kc + 1) * P, :])
            yield dma_all

            for j in range(2):
                for kc in range(8):
                    def mm(kc=kc, j=j):
                        if kc == 0:
                            pool = scp if on_act else fillp
                            st["ps"] = pool.tile([P, 1024], F32,
                                                 name=f"pj{mt}{j}", tag="mm")
                        for nb in range(2):
                            c0 = j * 1024 + nb * 512
                            nc.tensor.matmul(
                                st["ps"][:, nb * 512:(nb + 1) * 512],
                                lhsT=w_r[:, kc, mt * P:(mt + 1) * P],
                                rhs=st["xs"][kc][:, c0:c0 + 512],
                                start=(kc == 0), stop=(kc == 7),
                                skip_group_check=True,
                            )
                    yield mm

                def evict(j=j):
                    if kind != "v" and j == 0:
                        st["raw"] = raws.tile([P, L], BF,
                                              name=f"raw{kind}{mt}", tag="raw")
                    dst = vh_sb[mt] if kind == "v" else st["raw"]
                    dsl = dst[:, j * 1024:(j + 1) * 1024]
                    if on_act:
                        nc.scalar.activation(dsl, st["ps"][:], AF.Identity,
                                             bias=bias_sb[:, mt:mt + 1])
                    else:
                        nc.vector.scalar_tensor_tensor(
                            dsl, st["ps"][:], bias_sb[:, mt:mt + 1],
                            ones_sb[:], ALU.add, ALU.mult)
                yield evict

            if kind == "v":
                return

            cos_sb, sin_sb = (cq_sb, sq_sb) if kind == "q" else (ck_sb, sk_sb)

            def rope_dma():
                raw = st["raw"]
                rot = rots.tile([P, L], BF, name=f"rot{kind}{mt}", tag="rot")
                st["rot"] = rot
                for hb in range(2):
                    r0 = hb * DK
                    nc.sync.dma_start(rot[r0:r0 + 32, :],
                                      raw[r0 + 32:r0 + 64, :])
                    nc.sync.dma_start(rot[r0 + 32:r0 + 64, :],
                                      raw[r0:r0 + 32, :])
            yield rope_dma

            def rope_mul1():
                nc.vector.tensor_mul(st["raw"][:], st["raw"][:], cos_sb[:])
            yield rope_mul1

            def rope_mul2():
                nc.vector.tensor_mul(st["rot"][:], st["rot"][:], sin_sb[:])
                nc.vector.tensor_add(st["rot"][:], st["rot"][:], st["raw"][:])
            yield rope_mul2

            if kind == "q":
                for hl in range(2):
                    def bq_(hl=hl):
                        h = 2 * mt + hl
                        src = st["rot"][hl * DK:(hl + 1) * DK, :]
                        nc.vector.tensor_copy(qs2[h][0:DK, :], src)
                        nc.vector.tensor_copy(qs2[h][DK:P, :], src)
                    yield bq_
            else:
                for hl in range(2):
                    def bk_(hl=hl):
                        h = 2 * mt + hl
                        src = st["rot"][hl * DK:(hl + 1) * DK, :].rearrange(
                            "p (c g u) -> p c g u", g=2, u=DK)
                        dst = kh2[h].rearrange("p (c g u) -> p c g u",
                                               g=2, u=DK)
                        nc.vector.tensor_copy(dst[0:DK, :, 0, :],
                                              src[:, :, 0, :])
                        nc.vector.tensor_copy(dst[DK:P, :, 1, :],
                                              src[:, :, 1, :])
                    yield bk_

        def gen_vaug(mt, pool):
            """PE-transpose vh [dims, kt] -> vaug [kt, dims] (+ones col)."""
            dsts = [vaug[2 * mt + hl].rearrange("p (c u) -> p c u", u=65)
                    for hl in range(2)]
            for c in range(16):
                def tr(c=c):
                    pt = pool.tile([P, P], BF, name="pt", tag="mm")
                    nc.tensor.transpose(
                        pt[:], vh_sb[mt][:, c * P:(c + 1) * P], ident[:])
                    for hl in range(2):
                        nc.vector.tensor_copy(
                            dsts[hl][:, c, 0:DK], pt[:, hl * DK:(hl + 1) * DK])
                yield tr

        def gen_oproj(tiles, pool, alt_act=False):
            """Out-projection for the given token tiles."""
            for tb in tiles:
                def mm(tb=tb):
                    t0 = tb * P
                    po = pool.tile([P, D], F32, name="po", tag="mm")
                    for mt in range(NMT):
                        for nb in range(2):
                            nc.tensor.matmul(
                                po[:, nb * 512:(nb + 1) * 512],
                                lhsT=ctx_sb[mt][:, t0:t0 + P],
                                rhs=wo_sb[mt][:, nb * 512:(nb + 1) * 512],
                                start=(mt == 0), stop=(mt == NMT - 1),
                                skip_group_check=True,
                            )
                    ob = outs.tile([P, D], BF, name="ob", tag="out")
                    if alt_act and tb % 2 == 1:
                        nc.scalar.activation(ob[:], po[:], AF.Identity)
                    else:
                        nc.vector.tensor_copy(ob[:], po[:])
                    nc.sync.dma_start(outp[t0:t0 + P, :], ob[:])
                yield mm

        # ---------- attention with interleaved fill steps ----------
        fills = []

        def pump(n):
            for _ in range(n):
                if fills:
                    fills.pop(0)()

        def attention(h, j):
            """Head h, query block j (1024 tokens): scores/exp/ctx."""
            qs = qs2[h][:, j * 1024:(j + 1) * 1024]
            cp = cpp.tile([65, 1024], F32, name="cp", tag="cp")
            ex_prev = None
            for c in range(17):
                ex_cur = None
                if c < 16:
                    sc = scp.tile([P, 1024], F32, name="sc", tag="mm")
                    for nb in range(2):
                        nc.tensor.matmul(
                            sc[:, nb * 512:(nb + 1) * 512],
                            lhsT=kh2[h][:, c * P:(c + 1) * P],
                            rhs=qs[:, nb * 512:(nb + 1) * 512],
                            start=True, stop=True, skip_group_check=True,
                        )
                    ex_cur = expp.tile([P, 1024], BF, name="ex", tag="exp")
                    nc.scalar.activation(ex_cur[:], sc[:], AF.Exp)
                    if debug_dumps and h == 0 and j == 0 and c == 0:
                        nc.sync.dma_start(dbg["dbg_ex"][:], ex_cur[:])
                if c >= 1:
                    cpv = c - 1
                    for nb in range(2):
                        sl = slice(nb * 512, (nb + 1) * 512)
                        nc.tensor.matmul(
                            cp[:, sl],
                            lhsT=vaug[h][:, cpv * 65:(cpv + 1) * 65],
                            rhs=ex_prev[:, sl],
                            start=(cpv == 0), stop=(cpv == 15),
                            skip_group_check=True,
                        )
                pump(2)
                ex_prev = ex_cur
            # normalize: ctx_sb rows = ctx_unnorm * broadcast(1/den)
            cfull = smalls.tile([65, 1024], F32, name="cfull", tag="cfull")
            nc.vector.tensor_copy(cfull[:], cp[:])
            den = smalls.tile([1, 1024], F32, name="den", tag="den")
            nc.sync.dma_start(den[:], cfull[64:65, :])
            rec = smalls.tile([1, 1024], F32, name="rec", tag="rec")
            nc.vector.reciprocal_approx_fast(rec[:], den[:])
            bcs = smalls.tile([DK, 1024], F32, name="bcs", tag="bcs")
            nc.gpsimd.partition_broadcast(bcs[:], rec[:], channels=DK)
            if debug_dumps and h == 0 and j == 0:
                nc.sync.dma_start(dbg["dbg_cfull"][:], cfull[:])
                nc.sync.dma_start(dbg["dbg_rec"][:], rec[:])
                nc.sync.dma_start(dbg["dbg_bcs"][:], bcs[:])
            mt, hl = h // 2, h % 2
            nc.vector.tensor_mul(
                ctx_sb[mt][hl * DK:(hl + 1) * DK, j * 1024:(j + 1) * 1024],
                cfull[0:DK, :], bcs[:])

        # ---------- program ----------
        # pre-phase: first head-pair projections + v/vaug, PE-dense
        for step in gen_proj(qT, wq_sb, bq_sb, 0, "q", on_act=True):
            step()
        for step in gen_proj(kT, wk_sb, bk_sb, 0, "k", on_act=True):
            step()
        for step in gen_proj(vT, wv_sb, bv_sb, 0, "v", on_act=True):
            step()
        for step in gen_vaug(0, scp):
            step()

        # fills consumed inside the attention chunk loops (order matters:
        # vh[1]/vaug[1] before h2 ctx, qs2/kh2 h2,h3 before h2 scores)
        fills.extend(gen_proj(vT, wv_sb, bv_sb, 1, "v", on_act=False))
        fills.extend(gen_vaug(1, fillp))
        fills.extend(gen_proj(qT, wq_sb, bq_sb, 1, "q", on_act=False))
        fills.extend(gen_proj(kT, wk_sb, bk_sb, 1, "k", on_act=False))

        attention(0, 0)
        attention(1, 0)
        attention(0, 1)
        attention(1, 1)
        # drain any remaining projection fills before h2 needs them
        pump(len(fills))
        attention(2, 0)
        attention(3, 0)
        fills.extend(gen_oproj(range(0, 8), fillp))
        attention(2, 1)
        attention(3, 1)
        pump(len(fills))
        for step in gen_oproj(range(8, 16), scp, alt_act=True):
            step()

        if debug_dumps:
            nc.sync.dma_start(dbg["dbg_vaug"][:], vaug[0][:])
            nc.sync.dma_start(dbg["dbg_qs2"][:], qs2[0][:])
            nc.sync.dma_start(dbg["dbg_kh2"][:], kh2[0][:])
            nc.sync.dma_start(dbg["dbg_vh"][:], vh_sb[0][:])
            nc.sync.dma_start(dbg["dbg_ctx"][:], ctx_sb[0][:])

    return nc


def _rope_tables():
    """Host-built RoPE tables, transposed to [d, t], 2 heads stacked.

    sin is sign-folded for the rotate-half convention; q tables carry the
    1/sqrt(dk) attention scale.
    """
    inv_freq = 1.0 / (ROPE_BASE ** (np.arange(0, DK, 2, dtype=np.float64) / DK))
    t = np.arange(L, dtype=np.float64)
    ang = np.outer(t, inv_freq)               # [L, 32]
    emb = np.concatenate([ang, ang], axis=1)  # [L, 64]
    cos = np.cos(emb).T.astype(np.float32)    # [64, L]
    sin = np.sin(emb).T.astype(np.float32)
    sin_folded = sin.copy()
    sin_folded[:32] *= -1.0
    scale = 1.0 / np.sqrt(DK)
    cos2 = np.concatenate([cos, cos], axis=0)                # [128, L]
    sin2 = np.concatenate([sin_folded, sin_folded], axis=0)  # [128, L]
    bf = ml_dtypes.bfloat16
    return (
        (cos2 * scale).astype(bf), (sin2 * scale).astype(bf),
        cos2.astype(bf), sin2.astype(bf),
    )


_NC_CACHE = {}


def _get_nc():
    if "nc" not in _NC_CACHE:
        nc = build_nc()
        nc.finalize()
        _NC_CACHE["nc"] = nc
    return _NC_CACHE["nc"]


def make_in_maps(q, k, v, Wq, bq, Wk, bk, Wv, bv, Wo, bo):
    bf = ml_dtypes.bfloat16
    cos_q, sin_q, cos_k, sin_k = _rope_tables()
    xT = {}
    for b in range(B):
        xT[("q", b)] = np.ascontiguousarray(np.asarray(q)[b].T).astype(bf)
        xT[("k", b)] = np.ascontiguousarray(np.asarray(k)[b].T).astype(bf)
        xT[("v", b)] = np.ascontiguousarray(np.asarray(v)[b].T).astype(bf)
    in_maps = []
    for c in range(NCORES):
        b, g = c // 4, c % 4
        hs = slice(g * PD, (g + 1) * PD)
        in_maps.append({
            "qT": xT[("q", b)], "kT": xT[("k", b)], "vT": xT[("v", b)],
            "wqT": np.ascontiguousarray(np.asarray(Wq)[hs, :].T).astype(bf),
            "wkT": np.ascontiguousarray(np.asarray(Wk)[hs, :].T).astype(bf),
            "wvT": np.ascontiguousarray(np.asarray(Wv)[hs, :].T).astype(bf),
            "woT": np.ascontiguousarray(np.asarray(Wo)[:, hs].T).astype(bf),
            "bq": np.asarray(bq[hs], np.float32).reshape(PD, 1),
            "bk": np.asarray(bk[hs], np.float32).reshape(PD, 1),
            "bv": np.asarray(bv[hs], np.float32).reshape(PD, 1),
            "cos_q": cos_q, "sin_q": sin_q, "cos_k": cos_k, "sin_k": sin_k,
        })
    return in_maps


def kernel(q, k, v, Wq, bq, Wk, bk, Wv, bv, Wo, bo):
    assert q.shape == (B, L, D) and k.shape == (B, L, D) and v.shape == (B, L, D)
    in_maps = make_in_maps(q, k, v, Wq, bq, Wk, bk, Wv, bv, Wo, bo)
    nc = _get_nc()
    res = run_bass_kernel_spmd(nc, in_maps, list(range(NCORES)))
    out = np.zeros((B, TOK, D), np.float64)
    for c, r in enumerate(res.results):
        out[c // 4] += r["outp"].astype(np.float64)
    out += np.asarray(bo, np.float64)[None, None, :]
    return out.astype(np.float32)
